# revision 11
# baseline (speedup 1.0000x reference)
"""Trainium2 Bass kernel for nn_CrossAttention (B=2, Tq=Tk=2048, D=1024, H=16).

Sharding: 8 cores; core c owns batch b = c // 4 and query rows
[512*(c%4), 512*(c%4+1)) of that batch. Each core computes the full
attention + projections for its query slice (all 16 heads), so the
unshard is a pure concat. No collectives.

Device layout is fully "transposed" so no on-chip transposes are needed:
  - host feeds q^T and kv^T (plus bf16-cast weights)
  - Q^T[do, t]  = sum_di Wq[di, do] * q^T[di, t]        (lhsT=Wq chunk)
  - K^T[ko, k]  likewise from kv^T
  - V[k, dv]    = sum_di kv^T[di, k]^T ... (lhsT=kv^T chunk, rhs=Wkv_v)
  - S^T[k, q]   = sum_d K^T[d, k]^T ... (lhsT=K^T chunk, rhs=Q^T) ; d=64
  - P^T         = exp(S^T * 1/8 + mask_bias)   (ACT, bf16 out)
  - O^T[d, q] & rowsum = matmul with stationary [V_h | ones] (M=65)
  - Y[q, n]     = sum_m O^T[m, q]^T ... (lhsT=O^T chunk, rhs=Wo chunk)

The key-padding mask becomes a per-position additive bias (-80 for
masked) applied inside the exp activation; key chunks of 128 that are
fully masked for every batch are dropped on the host (compacted k axis),
which also shrinks the K/V projections and the whole attention loop.
"""

import numpy as np
import ml_dtypes

import concourse.bass as bass
import concourse.mybir as mybir
import concourse.tile as tile
from concourse import bacc
from concourse.bass_utils import run_bass_kernel_spmd
from concourse.bass_interp import get_hw_module

B, TQ, TK, D, H = 2, 2048, 2048, 1024, 16
HD = D // H  # 64
N_CORES = 8
QLOC = (B * TQ) // N_CORES  # 512 query rows per core
SCALE = HD ** -0.5  # 0.125

F32 = mybir.dt.float32
BF16 = mybir.dt.bfloat16
Exp = mybir.ActivationFunctionType.Exp

_cache: dict[int, "bass.Bass"] = {}


def _build_program(n_kc: int, dbg: bool = False):
    """Build + compile the single-core program (SPMD across 8 cores).

    n_kc: number of active 128-wide key chunks (<= 16).
    """
    NK = n_kc * 128

    nc = bacc.Bacc("TRN2", target_bir_lowering=False, debug=False,
                   num_devices=N_CORES)
    if dbg:
        dbg_v = nc.dram_tensor("dbg_v", [128, n_kc, 16 * 65], BF16,
                               kind="ExternalOutput")
        dbg_ot = nc.dram_tensor("dbg_ot", [128, 8, QLOC], BF16,
                                kind="ExternalOutput")
        dbg_po = nc.dram_tensor("dbg_po", [128, QLOC], F32,
                                kind="ExternalOutput")
        dbg_rb = nc.dram_tensor("dbg_rb", [128, QLOC], F32,
                                kind="ExternalOutput")

    # ---- DRAM I/O (per-core shapes) ----
    qt_d = nc.dram_tensor("qt", [8, 128, QLOC], BF16, kind="ExternalInput")
    kvt_d = nc.dram_tensor("kvt", [8, 128, NK], BF16, kind="ExternalInput")
    wq_d = nc.dram_tensor("wq", [8, 128, D], BF16, kind="ExternalInput")
    wkk_d = nc.dram_tensor("wkk", [8, 128, D], BF16, kind="ExternalInput")
    wkv_d = nc.dram_tensor("wkv", [8, 128, D], BF16, kind="ExternalInput")
    wo_d = nc.dram_tensor("wo", [8, 128, D], BF16, kind="ExternalInput")
    bq_d = nc.dram_tensor("bq", [8, 128], F32, kind="ExternalInput")
    bkk_d = nc.dram_tensor("bkk", [8, 128], F32, kind="ExternalInput")
    bkv_d = nc.dram_tensor("bkv", [1, D], F32, kind="ExternalInput")
    bo_d = nc.dram_tensor("bo", [1, D], F32, kind="ExternalInput")
    biask_d = nc.dram_tensor("biask", [128, n_kc], F32, kind="ExternalInput")
    y_d = nc.dram_tensor("y", [QLOC, D], F32, kind="ExternalOutput")

    with tile.TileContext(nc) as tc:
        with (
            tc.tile_pool(name="const", bufs=1) as const,
            tc.tile_pool(name="persist", bufs=1) as persist,
            tc.tile_pool(name="ps", bufs=2, space="PSUM") as ps_pool,
            tc.tile_pool(name="ps_o", bufs=4, space="PSUM") as ps_o_pool,
            tc.tile_pool(name="work", bufs=4) as work,
            tc.tile_pool(name="norm", bufs=2) as norm_pool,
        ):
            # --- constants ---
            biask = const.tile([128, n_kc], F32)
            nc.sync.dma_start(biask[:], biask_d.ap())
            bq_sb = const.tile([128, 8], F32)
            nc.sync.dma_start(bq_sb[:], bq_d.ap().rearrange("c p -> p c"))
            bkk_sb = const.tile([128, 8], F32)
            nc.sync.dma_start(bkk_sb[:], bkk_d.ap().rearrange("c p -> p c"))
            bkv_bc = const.tile([128, D], F32)
            nc.sync.dma_start(bkv_bc[0:1, :], bkv_d.ap())
            nc.gpsimd.partition_broadcast(bkv_bc[:], bkv_bc[0:1, :])
            bo_bc = const.tile([128, D], F32)
            nc.sync.dma_start(bo_bc[0:1, :], bo_d.ap())
            nc.gpsimd.partition_broadcast(bo_bc[:], bo_bc[0:1, :])

            # --- persistent activations ---
            qtp = persist.tile([128, 8, QLOC], BF16)   # Q^T  [1024(do), 512]
            kt = persist.tile([128, 8, NK], BF16)      # K^T  [1024(ko), NK]
            v_sb = persist.tile([128, n_kc, 16 * 65], BF16)  # V+ones per head
            ot = persist.tile([128, 8, QLOC], BF16)    # O^T  [1024(m), 512]

            # ones columns of v_sb (col 64 of each 65-wide head block)
            nc.vector.memset(
                v_sb[:].rearrange("p k (h c) -> p k h c", c=65)[:, :, :, 64:65],
                1.0,
            )

            with (
                tc.tile_pool(name="wload", bufs=1) as wload,
                tc.tile_pool(name="inload", bufs=1) as inload,
            ):
                wq_sb = wload.tile([128, 8, D], BF16)
                wkk_sb = wload.tile([128, 8, D], BF16)
                wkv_sb = wload.tile([128, 8, D], BF16)
                qt_sb = inload.tile([128, 8, QLOC], BF16)
                kvt_sb = inload.tile([128, 8, NK], BF16)
                for di in range(8):
                    nc.sync.dma_start(wq_sb[:, di, :], wq_d.ap()[di])
                    nc.sync.dma_start(wkk_sb[:, di, :], wkk_d.ap()[di])
                    nc.sync.dma_start(wkv_sb[:, di, :], wkv_d.ap()[di])
                    nc.sync.dma_start(qt_sb[:, di, :], qt_d.ap()[di])
                    nc.sync.dma_start(kvt_sb[:, di, :], kvt_d.ap()[di])

                # ---- stage A: Q^T projection ----
                for do in range(8):
                    ps = ps_pool.tile([128, QLOC], F32, tag="ps")
                    for di in range(8):
                        nc.tensor.matmul(
                            ps[:], wq_sb[:, di, bass.ts(do, 128)],
                            qt_sb[:, di, :], start=(di == 0), stop=(di == 7),
                        )
                    nc.vector.tensor_scalar_add(
                        qtp[:, do, :], ps[:], bq_sb[:, do:do + 1])

                # ---- stage B: K^T projection ----
                nsplits = [(s, min(512, NK - s)) for s in range(0, NK, 512)]
                for ko in range(8):
                    for (s, w) in nsplits:
                        ps = ps_pool.tile([128, 512], F32, tag="ps")
                        for di in range(8):
                            nc.tensor.matmul(
                                ps[:, :w], wkk_sb[:, di, bass.ts(ko, 128)],
                                kvt_sb[:, di, s:s + w],
                                start=(di == 0), stop=(di == 7),
                            )
                        nc.vector.tensor_scalar_add(
                            kt[:, ko, s:s + w], ps[:, :w], bkk_sb[:, ko:ko + 1])

                # ---- stage C: V projection (natural [k, dv] layout) ----
                v_view = v_sb[:].rearrange("p k (h c) -> p k h c", c=65)
                bkv_view = bkv_bc[:].rearrange("p (h c) -> p h c", c=64)
                for kc in range(n_kc):
                    for dvc in range(2):
                        ps = ps_pool.tile([128, 512], F32, tag="ps")
                        for di in range(8):
                            nc.tensor.matmul(
                                ps[:], kvt_sb[:, di, bass.ts(kc, 128)],
                                wkv_sb[:, di, bass.ts(dvc, 512)],
                                start=(di == 0), stop=(di == 7),
                            )
                        nc.vector.tensor_tensor(
                            out=v_view[:, kc, 8 * dvc:8 * dvc + 8, 0:64],
                            in0=ps[:].rearrange("p (h c) -> p h c", c=64),
                            in1=bkv_view[:, 8 * dvc:8 * dvc + 8, :],
                            op=mybir.AluOpType.add,
                        )

            # ---- stage D: attention (per head pair, packed scores) ----
            for pair in range(8):
                po = []
                for sub in range(2):
                    po.append(ps_o_pool.tile([128, QLOC], F32, tag="ps_o",
                                             name=f"po_{pair}_{sub}"))
                for kc in range(n_kc):
                    for sub in range(2):
                        h = 2 * pair + sub
                        r0 = 64 * sub
                        pss = ps_pool.tile([128, QLOC], F32, tag="pss")
                        nc.tensor.matmul(
                            pss[:],
                            kt[r0:r0 + 64, pair, bass.ts(kc, 128)],
                            qtp[r0:r0 + 64, pair, :],
                            start=True, stop=True,
                        )
                        pt = work.tile([128, QLOC], BF16, tag="pt")
                        nc.scalar.activation(
                            pt[:], pss[:], Exp,
                            bias=biask[:, kc:kc + 1], scale=SCALE,
                        )
                        nc.tensor.matmul(
                            po[sub][0:65, :],
                            v_sb[:, kc, bass.ts(h, 65)],
                            pt[:],
                            start=(kc == 0), stop=(kc == n_kc - 1),
                        )
                for sub in range(2):
                    h = 2 * pair + sub
                    if dbg and pair == 0 and sub == 0:
                        po_cp = norm_pool.tile([128, QLOC], F32, tag="po_cp",
                                               bufs=1)
                        nc.vector.tensor_copy(po_cp[:], po[sub][:])
                        nc.sync.dma_start(dbg_po.ap(), po_cp[:])
                    rb = norm_pool.tile([128, QLOC], F32, tag="rb")
                    nc.vector.reciprocal(rb[64:65, :], po[sub][64:65, :])
                    rs0 = norm_pool.tile([1, QLOC], F32, tag="rs0")
                    nc.sync.dma_start(rs0[:], rb[64:65, :])
                    nc.gpsimd.partition_broadcast(
                        rb[0:64, :], rs0[0:1, :], channels=64)
                    if dbg and pair == 0 and sub == 0:
                        nc.sync.dma_start(dbg_rb.ap(), rb[:])
                    nt = norm_pool.tile([64, QLOC], BF16, tag="nt")
                    nc.vector.tensor_tensor(
                        out=nt[:], in0=po[sub][0:64, :], in1=rb[0:64, :],
                        op=mybir.AluOpType.mult,
                    )
                    nc.sync.dma_start(ot[64 * sub:64 * sub + 64, pair, :], nt[:])

            if dbg:
                nc.sync.dma_start(dbg_v.ap(), v_sb[:])
                nc.sync.dma_start(dbg_ot.ap(), ot[:])

            # ---- stage E: output projection ----
            with tc.tile_pool(name="wo_pool", bufs=1) as wo_pool:
                wo_sb = wo_pool.tile([128, 8, D], BF16)
                for mc in range(8):
                    nc.sync.dma_start(wo_sb[:, mc, :], wo_d.ap()[mc])
                for qm in range(QLOC // 128):
                    y_sb = work.tile([128, D], F32, tag="y")
                    for nn in range(2):
                        ps = ps_pool.tile([128, 512], F32, tag="ps")
                        for mc in range(8):
                            nc.tensor.matmul(
                                ps[:], ot[:, mc, bass.ts(qm, 128)],
                                wo_sb[:, mc, bass.ts(nn, 512)],
                                start=(mc == 0), stop=(mc == 7),
                            )
                        nc.vector.tensor_tensor(
                            out=y_sb[:, bass.ts(nn, 512)], in0=ps[:],
                            in1=bo_bc[:, bass.ts(nn, 512)],
                            op=mybir.AluOpType.add,
                        )
                    nc.sync.dma_start(y_d.ap()[bass.ts(qm, 128), :], y_sb[:])

    nc.compile()
    nc.m = get_hw_module(nc.m)
    return nc


def _get_program(n_kc: int):
    if n_kc not in _cache:
        _cache[n_kc] = _build_program(n_kc)
    return _cache[n_kc]


def _to_bf16(x):
    return np.ascontiguousarray(x).astype(ml_dtypes.bfloat16)


def kernel(q, kv, key_padding_mask, Wq, bq, Wkv, bkv, Wo, bo):
    q = np.asarray(q, dtype=np.float32)
    kv = np.asarray(kv, dtype=np.float32)
    mask = np.asarray(key_padding_mask).astype(bool)
    Wq = np.asarray(Wq, dtype=np.float32)
    bq = np.asarray(bq, dtype=np.float32)
    Wkv = np.asarray(Wkv, dtype=np.float32)
    bkv = np.asarray(bkv, dtype=np.float32)
    Wo = np.asarray(Wo, dtype=np.float32)
    bo = np.asarray(bo, dtype=np.float32)

    # --- active key chunks (a chunk is kept if any batch has a live key) ---
    live = ~mask  # [B, TK], True = real key
    chunk_live = live.reshape(B, TK // 128, 128).any(axis=2).any(axis=0)
    active = np.flatnonzero(chunk_live)  # chunk ids, ascending
    n_kc = int(len(active))
    assert n_kc >= 1
    NK = n_kc * 128

    nc = _get_program(n_kc)

    # --- shared (per-core-identical) weight prep ---
    wq_h = _to_bf16(Wq).reshape(8, 128, D)
    wkk_h = _to_bf16(Wkv[:, :D]).reshape(8, 128, D)
    wkv_h = _to_bf16(Wkv[:, D:]).reshape(8, 128, D)
    wo_h = _to_bf16(Wo).reshape(8, 128, D)
    bq_h = bq.reshape(8, 128)
    bkk_h = bkv[:D].reshape(8, 128)
    bkv_h = bkv[D:].reshape(1, D)
    bo_h = bo.reshape(1, D)

    shared = {
        "wq": wq_h, "wkk": wkk_h, "wkv": wkv_h, "wo": wo_h,
        "bq": bq_h, "bkk": bkk_h, "bkv": bkv_h, "bo": bo_h,
    }

    # --- per-core inputs ---
    sel = (active[:, None] * 128 + np.arange(128)[None, :]).reshape(-1)  # [NK]
    in_maps = []
    for c in range(N_CORES):
        b = c // 4
        r0 = (c % 4) * QLOC
        qt = _to_bf16(q[b, r0:r0 + QLOC, :].T).reshape(8, 128, QLOC)
        kvt = _to_bf16(kv[b][sel, :].T).reshape(8, 128, NK)
        bias_flat = np.where(mask[b][sel], np.float32(-80.0), np.float32(0.0))
        biask = np.ascontiguousarray(
            bias_flat.reshape(n_kc, 128).T).astype(np.float32)
        m = dict(shared)
        m.update({"qt": qt, "kvt": kvt, "biask": biask})
        in_maps.append(m)

    res = run_bass_kernel_spmd(
        nc, in_maps, core_ids=list(range(N_CORES)), trace=False)

    out = np.empty((B, TQ, D), dtype=np.float32)
    for c in range(N_CORES):
        b = c // 4
        r0 = (c % 4) * QLOC
        out[b, r0:r0 + QLOC, :] = res.results[c]["y"]
    return out


# revision 17
# speedup vs baseline: 1.0361x; 1.0361x over previous
"""Trainium2 Bass kernel for nn_CrossAttention (B=2, Tq=Tk=2048, D=1024, H=16).

Sharding: 8 cores; core c owns batch b = c // 4 and query rows
[512*(c%4), 512*(c%4+1)) of that batch. Each core computes the full
attention + projections for its query slice (all 16 heads), so the
unshard is a pure concat. No collectives.

Device layout is fully "transposed" so no on-chip transposes are needed:
  - host feeds q^T and kv^T (plus bf16-cast weights)
  - Q^T[do, t]  = sum_di Wq[di, do] * q^T[di, t]        (lhsT=Wq chunk)
  - K^T[ko, k]  likewise from kv^T
  - V[k, dv]    = sum_di kv^T[di, k]^T ... (lhsT=kv^T chunk, rhs=Wkv_v)
  - S^T[k, q]   = sum_d K^T[d, k]^T ... (lhsT=K^T chunk, rhs=Q^T) ; d=64
  - P^T         = exp(S^T * 1/8 + mask_bias)   (ACT, bf16 out)
  - O^T[d, q] & rowsum = matmul with stationary [V_h | ones] (M=65)
  - Y[q, n]     = sum_m O^T[m, q]^T ... (lhsT=O^T chunk, rhs=Wo chunk)

The key-padding mask becomes a per-position additive bias (-80 for
masked) applied inside the exp activation; key chunks of 128 that are
fully masked for every batch are dropped on the host (compacted k axis),
which also shrinks the K/V projections and the whole attention loop.
"""

import numpy as np
import ml_dtypes

import concourse.bass as bass
import concourse.mybir as mybir
import concourse.tile as tile
from concourse import bacc
from concourse.bass_utils import run_bass_kernel_spmd
from concourse.bass_interp import get_hw_module

B, TQ, TK, D, H = 2, 2048, 2048, 1024, 16
HD = D // H  # 64
N_CORES = 8
QLOC = (B * TQ) // N_CORES  # 512 query rows per core
SCALE = HD ** -0.5  # 0.125

F32 = mybir.dt.float32
BF16 = mybir.dt.bfloat16
Exp = mybir.ActivationFunctionType.Exp

_cache: dict[int, "bass.Bass"] = {}


def _build_program(n_kc: int, dbg: bool = False):
    """Build + compile the single-core program (SPMD across 8 cores).

    n_kc: number of active 128-wide key chunks (<= 16).
    """
    NK = n_kc * 128

    nc = bacc.Bacc("TRN2", target_bir_lowering=False, debug=False,
                   num_devices=N_CORES)
    if dbg:
        dbg_v = nc.dram_tensor("dbg_v", [128, n_kc, 16 * 65], BF16,
                               kind="ExternalOutput")
        dbg_ot = nc.dram_tensor("dbg_ot", [128, 8, QLOC], BF16,
                                kind="ExternalOutput")
        dbg_po = nc.dram_tensor("dbg_po", [128, QLOC], F32,
                                kind="ExternalOutput")
        dbg_rb = nc.dram_tensor("dbg_rb", [128, QLOC], F32,
                                kind="ExternalOutput")

    # ---- DRAM I/O (per-core shapes) ----
    qt_d = nc.dram_tensor("qt", [8, 128, QLOC], BF16, kind="ExternalInput")
    kvt_d = nc.dram_tensor("kvt", [8, 128, NK], BF16, kind="ExternalInput")
    wq_d = nc.dram_tensor("wq", [8, 128, D], BF16, kind="ExternalInput")
    wkk_d = nc.dram_tensor("wkk", [8, 128, D], BF16, kind="ExternalInput")
    wkv_d = nc.dram_tensor("wkv", [8, 128, D], BF16, kind="ExternalInput")
    wo_d = nc.dram_tensor("wo", [8, 128, D], BF16, kind="ExternalInput")
    bq_d = nc.dram_tensor("bq", [8, 128], F32, kind="ExternalInput")
    bkk_d = nc.dram_tensor("bkk", [8, 128], F32, kind="ExternalInput")
    bkv_d = nc.dram_tensor("bkv", [1, D], F32, kind="ExternalInput")
    bo_d = nc.dram_tensor("bo", [1, D], F32, kind="ExternalInput")
    biask_d = nc.dram_tensor("biask", [128, n_kc], F32, kind="ExternalInput")
    y_d = nc.dram_tensor("y", [QLOC, D], F32, kind="ExternalOutput")

    with tile.TileContext(nc) as tc:
        with (
            tc.tile_pool(name="const", bufs=1) as const,
            tc.tile_pool(name="persist", bufs=1) as persist,
            tc.tile_pool(name="ps", bufs=2, space="PSUM") as ps_pool,
            tc.tile_pool(name="ps_o", bufs=4, space="PSUM") as ps_o_pool,
            tc.tile_pool(name="work", bufs=4) as work,
            tc.tile_pool(name="norm", bufs=2) as norm_pool,
        ):
            # --- constants ---
            biask = const.tile([128, n_kc], F32)
            nc.sync.dma_start(biask[:], biask_d.ap())
            bq_sb = const.tile([128, 8], F32)
            nc.sync.dma_start(bq_sb[:], bq_d.ap().rearrange("c p -> p c"))
            bkk_sb = const.tile([128, 8], F32)
            nc.sync.dma_start(bkk_sb[:], bkk_d.ap().rearrange("c p -> p c"))
            bkv_bc = const.tile([128, D], F32)
            nc.sync.dma_start(bkv_bc[0:1, :], bkv_d.ap())
            nc.gpsimd.partition_broadcast(bkv_bc[:], bkv_bc[0:1, :])
            bo_bc = const.tile([128, D], F32)
            nc.sync.dma_start(bo_bc[0:1, :], bo_d.ap())
            nc.gpsimd.partition_broadcast(bo_bc[:], bo_bc[0:1, :])

            # --- persistent activations ---
            qtp = persist.tile([128, 8, QLOC], BF16)   # Q^T  [1024(do), 512]
            kt = persist.tile([128, 8, NK], BF16)      # K^T  [1024(ko), NK]
            v_sb = persist.tile([128, n_kc, 16 * 65], BF16)  # V+ones per head
            ot = persist.tile([128, 8, QLOC], BF16)    # O^T  [1024(m), 512]

            # ones columns of v_sb (col 64 of each 65-wide head block)
            nc.vector.memset(
                v_sb[:].rearrange("p k (h c) -> p k h c", c=65)[:, :, :, 64:65],
                1.0,
            )

            with (
                tc.tile_pool(name="wload", bufs=1) as wload,
                tc.tile_pool(name="inload", bufs=1) as inload,
            ):
                wq_sb = wload.tile([128, 8, D], BF16)
                wkk_sb = wload.tile([128, 8, D], BF16)
                wkv_sb = wload.tile([128, 8, D], BF16)
                qt_sb = inload.tile([128, 8, QLOC], BF16)
                kvt_sb = inload.tile([128, 8, NK], BF16)
                # stage-A inputs first so PE can start immediately
                for di in range(8):
                    nc.sync.dma_start(qt_sb[:, di, :], qt_d.ap()[di])
                    nc.sync.dma_start(wq_sb[:, di, :], wq_d.ap()[di])
                for di in range(8):
                    nc.sync.dma_start(kvt_sb[:, di, :], kvt_d.ap()[di])
                    nc.sync.dma_start(wkk_sb[:, di, :], wkk_d.ap()[di])
                    nc.sync.dma_start(wkv_sb[:, di, :], wkv_d.ap()[di])

                # ---- stage A: Q^T projection ----
                for do in range(8):
                    ps = ps_pool.tile([128, QLOC], F32, tag="ps")
                    for di in range(8):
                        nc.tensor.matmul(
                            ps[:], wq_sb[:, di, bass.ts(do, 128)],
                            qt_sb[:, di, :], start=(di == 0), stop=(di == 7),
                        )
                    nc.vector.tensor_scalar_add(
                        qtp[:, do, :], ps[:], bq_sb[:, do:do + 1])

                # ---- stage B: K^T projection ----
                nsplits = [(s, min(512, NK - s)) for s in range(0, NK, 512)]
                for ko in range(8):
                    for (s, w) in nsplits:
                        ps = ps_pool.tile([128, 512], F32, tag="ps")
                        for di in range(8):
                            nc.tensor.matmul(
                                ps[:, :w], wkk_sb[:, di, bass.ts(ko, 128)],
                                kvt_sb[:, di, s:s + w],
                                start=(di == 0), stop=(di == 7),
                            )
                        nc.vector.tensor_scalar_add(
                            kt[:, ko, s:s + w], ps[:, :w], bkk_sb[:, ko:ko + 1])

                # ---- stage C: V projection (natural [k, dv] layout) ----
                v_view = v_sb[:].rearrange("p k (h c) -> p k h c", c=65)
                bkv_view = bkv_bc[:].rearrange("p (h c) -> p h c", c=64)
                for kc in range(n_kc):
                    for dvc in range(2):
                        ps = ps_pool.tile([128, 512], F32, tag="ps")
                        for di in range(8):
                            nc.tensor.matmul(
                                ps[:], kvt_sb[:, di, bass.ts(kc, 128)],
                                wkv_sb[:, di, bass.ts(dvc, 512)],
                                start=(di == 0), stop=(di == 7),
                            )
                        nc.vector.tensor_tensor(
                            out=v_view[:, kc, 8 * dvc:8 * dvc + 8, 0:64],
                            in0=ps[:].rearrange("p (h c) -> p h c", c=64),
                            in1=bkv_view[:, 8 * dvc:8 * dvc + 8, :],
                            op=mybir.AluOpType.add,
                        )

            # ---- stage D: attention (per head pair, packed scores) ----
            wo_cm = tc.tile_pool(name="wo_pool", bufs=1)
            wo_pool = wo_cm.__enter__()
            wo_sb = wo_pool.tile([128, 8, D], BF16)
            for mc in range(8):
                nc.sync.dma_start(wo_sb[:, mc, :], wo_d.ap()[mc])
            for pair in range(8):
                po = []
                for sub in range(2):
                    po.append(ps_o_pool.tile([128, QLOC], F32, tag="ps_o",
                                             name=f"po_{pair}_{sub}"))
                for kc in range(n_kc):
                    for sub in range(2):
                        h = 2 * pair + sub
                        r0 = 64 * sub
                        pss = ps_pool.tile([128, QLOC], F32, tag="pss")
                        nc.tensor.matmul(
                            pss[:],
                            kt[r0:r0 + 64, pair, bass.ts(kc, 128)],
                            qtp[r0:r0 + 64, pair, :],
                            start=True, stop=True,
                        )
                        pt = work.tile([128, QLOC], BF16, tag="pt")
                        nc.scalar.activation(
                            pt[:], pss[:], Exp,
                            bias=biask[:, kc:kc + 1], scale=SCALE,
                        )
                        nc.tensor.matmul(
                            po[sub][0:65, :],
                            v_sb[:, kc, bass.ts(h, 65)],
                            pt[:],
                            start=(kc == 0), stop=(kc == n_kc - 1),
                        )
                for sub in range(2):
                    h = 2 * pair + sub
                    if dbg and pair == 0 and sub == 0:
                        po_cp = norm_pool.tile([128, QLOC], F32, tag="po_cp",
                                               bufs=1)
                        nc.vector.tensor_copy(po_cp[:], po[sub][:])
                        nc.sync.dma_start(dbg_po.ap(), po_cp[:])
                    rb = norm_pool.tile([128, QLOC], F32, tag="rb")
                    nc.vector.reciprocal(rb[64:65, :], po[sub][64:65, :])
                    rs0 = norm_pool.tile([1, QLOC], F32, tag="rs0")
                    nc.sync.dma_start(rs0[:], rb[64:65, :])
                    nc.gpsimd.partition_broadcast(
                        rb[0:64, :], rs0[0:1, :], channels=64)
                    if dbg and pair == 0 and sub == 0:
                        nc.sync.dma_start(dbg_rb.ap(), rb[:])
                    nt = norm_pool.tile([64, QLOC], BF16, tag="nt")
                    nc.vector.tensor_tensor(
                        out=nt[:], in0=po[sub][0:64, :], in1=rb[0:64, :],
                        op=mybir.AluOpType.mult,
                    )
                    nc.sync.dma_start(ot[64 * sub:64 * sub + 64, pair, :], nt[:])

            if dbg:
                nc.sync.dma_start(dbg_v.ap(), v_sb[:])
                nc.sync.dma_start(dbg_ot.ap(), ot[:])

            # ---- stage E: output projection ----
            try:
                for qm in range(QLOC // 128):
                    y_sb = work.tile([128, D], F32, tag="y")
                    for nn in range(2):
                        ps = ps_pool.tile([128, 512], F32, tag="ps")
                        for mc in range(8):
                            nc.tensor.matmul(
                                ps[:], ot[:, mc, bass.ts(qm, 128)],
                                wo_sb[:, mc, bass.ts(nn, 512)],
                                start=(mc == 0), stop=(mc == 7),
                            )
                        nc.vector.tensor_tensor(
                            out=y_sb[:, bass.ts(nn, 512)], in0=ps[:],
                            in1=bo_bc[:, bass.ts(nn, 512)],
                            op=mybir.AluOpType.add,
                        )
                    nc.sync.dma_start(y_d.ap()[bass.ts(qm, 128), :], y_sb[:])
            finally:
                wo_cm.__exit__(None, None, None)

    nc.compile()
    nc.m = get_hw_module(nc.m)
    return nc


def _get_program(n_kc: int):
    if n_kc not in _cache:
        _cache[n_kc] = _build_program(n_kc)
    return _cache[n_kc]


def _to_bf16(x):
    return np.ascontiguousarray(x).astype(ml_dtypes.bfloat16)


def kernel(q, kv, key_padding_mask, Wq, bq, Wkv, bkv, Wo, bo):
    q = np.asarray(q, dtype=np.float32)
    kv = np.asarray(kv, dtype=np.float32)
    mask = np.asarray(key_padding_mask).astype(bool)
    Wq = np.asarray(Wq, dtype=np.float32)
    bq = np.asarray(bq, dtype=np.float32)
    Wkv = np.asarray(Wkv, dtype=np.float32)
    bkv = np.asarray(bkv, dtype=np.float32)
    Wo = np.asarray(Wo, dtype=np.float32)
    bo = np.asarray(bo, dtype=np.float32)

    # --- active key chunks (a chunk is kept if any batch has a live key) ---
    live = ~mask  # [B, TK], True = real key
    chunk_live = live.reshape(B, TK // 128, 128).any(axis=2).any(axis=0)
    active = np.flatnonzero(chunk_live)  # chunk ids, ascending
    n_kc = int(len(active))
    assert n_kc >= 1
    NK = n_kc * 128

    nc = _get_program(n_kc)

    # --- shared (per-core-identical) weight prep ---
    wq_h = _to_bf16(Wq).reshape(8, 128, D)
    wkk_h = _to_bf16(Wkv[:, :D]).reshape(8, 128, D)
    wkv_h = _to_bf16(Wkv[:, D:]).reshape(8, 128, D)
    wo_h = _to_bf16(Wo).reshape(8, 128, D)
    bq_h = bq.reshape(8, 128)
    bkk_h = bkv[:D].reshape(8, 128)
    bkv_h = bkv[D:].reshape(1, D)
    bo_h = bo.reshape(1, D)

    shared = {
        "wq": wq_h, "wkk": wkk_h, "wkv": wkv_h, "wo": wo_h,
        "bq": bq_h, "bkk": bkk_h, "bkv": bkv_h, "bo": bo_h,
    }

    # --- per-core inputs ---
    sel = (active[:, None] * 128 + np.arange(128)[None, :]).reshape(-1)  # [NK]
    in_maps = []
    for c in range(N_CORES):
        b = c // 4
        r0 = (c % 4) * QLOC
        qt = _to_bf16(q[b, r0:r0 + QLOC, :].T).reshape(8, 128, QLOC)
        kvt = _to_bf16(kv[b][sel, :].T).reshape(8, 128, NK)
        bias_flat = np.where(mask[b][sel], np.float32(-80.0), np.float32(0.0))
        biask = np.ascontiguousarray(
            bias_flat.reshape(n_kc, 128).T).astype(np.float32)
        m = dict(shared)
        m.update({"qt": qt, "kvt": kvt, "biask": biask})
        in_maps.append(m)

    res = run_bass_kernel_spmd(
        nc, in_maps, core_ids=list(range(N_CORES)), trace=False)

    out = np.empty((B, TQ, D), dtype=np.float32)
    for c in range(N_CORES):
        b = c // 4
        r0 = (c % 4) * QLOC
        out[b, r0:r0 + QLOC, :] = res.results[c]["y"]
    return out
